# revision 33
# baseline (speedup 1.0000x reference)
"""Trainium2 Bass kernel for nn_DifferentiableDenseHGTConv.

Self-contained: takes FULL inputs as numpy arrays, shards batch x target-row
halves across 8 NeuronCores (core c -> batch c//2, row-half c%2), runs one
SPMD Bass/Tile kernel, gathers the full [4,1024,256] fp32 output.

Per-core dataflow (I = 512 target rows, N = 1024 sources, H=8 heads, DK=32):
  All [N,N,H]-scale tensors live in [j(source)-partition, i(target)-free]
  orientation so the attention-weighted aggregation matmuls contract over j
  in PE partitions.  Per head the loop runs two pipelined passes over the 8
  source chunks (split in half-head groups to bound SBUF lifetimes):

  phase L: Sbar_r = Kbar^T Q~_r score pairs -> masked logits U_r = Sbar_r.m_r
           -> res -> exp -> attention numerator P;  row-sum gate and softmax
           1/Z are deferred to the epilogue as per-target diagonal scales.
           The attention weights W_r = P.m_r are issued on GPSIMD here so
           they are ready (off the critical path) when phase S needs them.

  phase S: the type-sum identity  sum_s S_sr = Sbar_r  lets aggregation run
           in the basis {Sbar_r -> V_2, S_0r -> V_0-V_2, S_1r -> V_1-V_2},
           reusing phase-L's U_r tiles for the Sbar channel (X = U.P) and
           computing only the s=0,1 score matmuls fresh (X = S.W_r).

  Elementwise work is spread across DVE (PSUM-direct mask muls, X muls),
  Act (exp + score drains) and GPSIMD (W muls).  Mask pairs m_01/m_23 are
  precomputed on the host.
"""
import math
import numpy as np
import ml_dtypes

T, R, H, DK = 3, 4, 8, 32
OUT = H * DK          # 256
IN = 256
B, N = 4, 1024
I = 512               # target rows per core
NCORES = 8
LN_EPS = 1e-5
BF16 = ml_dtypes.bfloat16

_built = {}


def _build_nc():
    """Build + compile the SPMD Bass module once per process."""
    if "nc" in _built:
        return _built["nc"]

    from contextlib import ExitStack
    import concourse.bass as bass
    import concourse.tile as tile
    from concourse import bacc, mybir
    from concourse.masks import make_identity

    dt = mybir.dt
    AF = mybir.ActivationFunctionType
    ALU = mybir.AluOpType

    nc = bacc.Bacc("TRN2", target_bir_lowering=False, debug=False,
                   enable_asserts=True, num_devices=NCORES)

    # ---------------- DRAM parameters (per core) ----------------
    xT = nc.declare_dram_parameter("xT", [IN + 1, N], dt.bfloat16, isOutput=False)
    xI = nc.declare_dram_parameter("xI", [I, IN], dt.bfloat16, isOutput=False)
    xTs = nc.declare_dram_parameter("xTs", [T, IN + 1, N], dt.bfloat16, isOutput=False)
    xTsI = nc.declare_dram_parameter("xTsI", [T, IN + 1, I], dt.bfloat16, isOutput=False)
    typesI = nc.declare_dram_parameter("typesI", [I, T], dt.float32, isOutput=False)
    mpd = nc.declare_dram_parameter("mpair", [N, 4 * I], dt.bfloat16, isOutput=False)
    wq = nc.declare_dram_parameter("wq", [T, IN + 1, OUT], dt.bfloat16, isOutput=False)
    wk = nc.declare_dram_parameter("wk", [T, IN + 1, OUT], dt.bfloat16, isOutput=False)
    wv = nc.declare_dram_parameter("wv", [T, IN + 1, OUT], dt.bfloat16, isOutput=False)
    abd = nc.declare_dram_parameter("abd", [R, OUT, OUT], dt.bfloat16, isOutput=False)
    bbd = nc.declare_dram_parameter("bbd", [R, OUT, OUT], dt.bfloat16, isOutput=False)
    wa = nc.declare_dram_parameter("wa", [T, OUT + 1, OUT], dt.bfloat16, isOutput=False)
    lng = nc.declare_dram_parameter("lng", [T, OUT], dt.bfloat16, isOutput=False)
    lnb = nc.declare_dram_parameter("lnb", [T, OUT], dt.bfloat16, isOutput=False)
    alph = nc.declare_dram_parameter("alph", [2, T], dt.float32, isOutput=False)
    outP = nc.declare_dram_parameter("out", [I, OUT], dt.float32, isOutput=True)

    def bcast(row_ap, parts=128):
        # DMA access pattern replicating one DRAM row across `parts` partitions
        return bass.AP(tensor=row_ap.tensor, offset=row_ap.offset,
                       ap=[[0, parts]] + list(row_ap.ap[1:]))

    def rep2(t, width):
        # free-dim repeat: read a [128, width] tile twice -> [128, 2*width]
        a = t[:, 0:width]
        return bass.AP(tensor=a.tensor, offset=a.offset,
                       ap=[list(a.ap[0]), [0, 2]] + list(a.ap[1:]))

    def stack3(param, row_lo, nrows, t_count, width):
        # AP over param[0:t_count, row_lo:row_lo+nrows, 0:width], dims
        # reordered to [rows(partition), t, width] for one merged DMA
        a = param[0:t_count, row_lo:row_lo + nrows, 0:width]
        ap = list(a.ap)
        return bass.AP(tensor=a.tensor, offset=a.offset,
                       ap=[list(ap[1]), list(ap[0]), list(ap[2])])

    def stack2(param, nrows, t_count, width):
        # AP over a 2D [t_count*nrows, width] param viewed [rows, t, width]
        a = param[0:nrows, 0:width]
        ap = list(a.ap)
        row_stride = ap[0][0]
        return bass.AP(tensor=a.tensor, offset=a.offset,
                       ap=[[row_stride, nrows], [row_stride * nrows, t_count],
                           list(ap[1])])

    NJC = N // 128    # 8 source chunks
    NIB = I // 128    # 4 target blocks
    OC = OUT // 128   # 2 output-dim chunks

    with ExitStack() as ctx:
        tc = ctx.enter_context(tile.TileContext(nc))
        const = ctx.enter_context(tc.tile_pool(name="const", bufs=1))
        work = ctx.enter_context(tc.tile_pool(name="work", bufs=3))

        dma_q = [nc.sync, nc.scalar]
        # ---------------- persistent constants ----------------
        ident = const.tile([128, 128], dt.bfloat16, tag="ident", name="ident")
        make_identity(nc, ident)
        onesCol = const.tile([128, 1], dt.bfloat16, tag="onescol", name="onescol")
        nc.vector.memset(onesCol, 1.0)
        onesRow = const.tile([1, I], dt.bfloat16, tag="onesrow", name="onesrow")
        nc.vector.memset(onesRow, 1.0)


        def load_w3(pool, param, name, klen=IN):
            # [T, klen+1, OUT] augmented weight, one merged DMA per k-chunk:
            # tiles [128, T*OUT] (+ [1, T*OUT] aug row); per-t views
            nkc = klen // 128
            chunks = []
            for kc in range(nkc):
                tl = pool.tile([128, T * OUT], dt.bfloat16, tag=f"{name}_{kc}",
                               name=f"{name}_{kc}")
                dma_q[kc % 2].dma_start(out=tl,
                                        in_=stack3(param, kc * 128, 128, T, OUT))
                chunks.append(tl)
            ta = pool.tile([1, T * OUT], dt.bfloat16, tag=f"{name}_a",
                           name=f"{name}_a")
            dma_q[nkc % 2].dma_start(out=ta, in_=stack3(param, klen, 1, T, OUT))
            out = []
            for t_ in range(T):
                tls = [chunks[kc][:, t_ * OUT:(t_ + 1) * OUT] for kc in range(nkc)]
                tls.append(ta[0:1, t_ * OUT:(t_ + 1) * OUT])
                out.append(tls)
            return out



        # Head-sliced PE operands are packed 3 heads per tile (32-row bands
        # at base partitions 0/32/64 -- the only legal operand bases).
        def hs(tiles, h, lo=None, hi=None):
            b = (h % 3) * 32
            if lo is None:
                return tiles[h // 3][b:b + 32, :]
            return tiles[h // 3][b:b + 32, lo:hi]

        def hpack_alloc(name_, width):
            return [const.tile([96, width], dt.bfloat16, tag=f"{name_}0", name=f"{name_}0"),
                    const.tile([96, width], dt.bfloat16, tag=f"{name_}1", name=f"{name_}1"),
                    const.tile([64, width], dt.bfloat16, tag=f"{name_}2", name=f"{name_}2")]

        def pack_dma(src, tiles, oc):
            # repack a [128, W] evac (heads oc*4..oc*4+3 in 32-row bands) into
            # the 3-per-tile head layout with two partition-contiguous DMAs
            if oc == 0:
                nc.sync.dma_start(out=tiles[0][0:96, :], in_=src[0:96, :])
                nc.scalar.dma_start(out=tiles[1][0:32, :], in_=src[96:128, :])
            else:
                nc.sync.dma_start(out=tiles[1][32:96, :], in_=src[0:64, :])
                nc.scalar.dma_start(out=tiles[2][0:64, :], in_=src[64:128, :])

        # ---------------- stage A: projections ----------------
        # Host pre-scales x by each soft-type column (aug row = types row), so
        # K~_s^T / QbarT / KbarT are plain augmented matmuls with PSUM
        # accumulation doing the type mixing.
        ktilT = []                                # s=0,1: 3 packed tiles [., N]
        kbarT = hpack_alloc("kbar", N)
        qtilT = []                                # per r: 3 packed tiles [., I]
        v2 = [None] * NJC                         # V_2 [N, 256]
        vd = [[None] * NJC for _ in range(2)]     # V_s - V_2 for s=0,1
        with tc.tile_pool(name="sA", bufs=1) as sA:
            wk_t = load_w3(sA, wk, "wk")
            abd_c = []
            for kc in range(OC):
                tl = sA.tile([128, R * OUT], dt.bfloat16, tag=f"abdc{kc}",
                             name=f"abdc{kc}")
                dma_q[kc % 2].dma_start(out=tl,
                                        in_=stack3(abd, kc * 128, 128, R, OUT))
                abd_c.append(tl)
            abd_t = [[abd_c[kc][:, r * OUT:(r + 1) * OUT] for kc in range(OC)]
                     for r in range(R)]

            # K side: K~T_s (s=0,1) and KbarT via PSUM accumulation over (s, kc)
            for s in range(2):
                ktilT.append(hpack_alloc(f"ktil{s}_", N))
            xsm = []
            for kc in range(2):
                tl = sA.tile([128, T * N], dt.bfloat16, tag=f"xsm{kc}",
                             name=f"xsm{kc}")
                dma_q[kc % 2].dma_start(out=tl,
                                        in_=stack3(xTs, kc * 128, 128, T, N))
                xsm.append(tl)
            tl = sA.tile([1, T * N], dt.bfloat16, tag="xsm2", name="xsm2")
            nc.sync.dma_start(out=tl, in_=stack3(xTs, IN, 1, T, N))
            xsm.append(tl)
            wq_t = load_w3(sA, wq, "wq")
            xim = []
            for kc in range(2):
                tl = sA.tile([128, T * I], dt.bfloat16, tag=f"xim{kc}",
                             name=f"xim{kc}")
                dma_q[kc % 2].dma_start(out=tl,
                                        in_=stack3(xTsI, kc * 128, 128, T, I))
                xim.append(tl)
            tl = sA.tile([1, T * I], dt.bfloat16, tag="xim2", name="xim2")
            nc.sync.dma_start(out=tl, in_=stack3(xTsI, IN, 1, T, I))
            xim.append(tl)
            xT_t = []
            for kc in range(2):
                tl = sA.tile([128, N], dt.bfloat16, tag=f"xT{kc}", name=f"xT{kc}")
                nc.sync.dma_start(out=tl, in_=xT[kc * 128:(kc + 1) * 128, :])
                xT_t.append(tl)
            xT_t.append(sA.tile([1, N], dt.bfloat16, tag="xT2", name="xT2"))
            nc.sync.dma_start(out=xT_t[2], in_=xT[IN:IN + 1, :])
            wv_t = load_w3(sA, wv, "wv")
            with tc.tile_pool(name="psK", bufs=3, space="PSUM") as psK:
                kbu = [sA.tile([128, N], dt.bfloat16, tag=f"kbu{oc}",
                               name=f"kbu{oc}") for oc in range(OC)]
                for s in range(T):
                    xs_t = [xsm[0][:, s * N:(s + 1) * N],
                            xsm[1][:, s * N:(s + 1) * N],
                            xsm[2][0:1, s * N:(s + 1) * N]]
                    for oc in range(OC):
                        pa = psK.tile([128, N], dt.float32, tag="pA", name="pA")
                        for kc in range(3):
                            for nh in range(2):
                                nsl = slice(nh * 512, (nh + 1) * 512)
                                nc.tensor.matmul(pa[:, nsl],
                                                 wk_t[s][kc][:, oc * 128:(oc + 1) * 128],
                                                 xs_t[kc][:, nsl],
                                                 start=(kc == 0), stop=(kc == 2))
                        kt = sA.tile([128, N], dt.bfloat16, tag="ktev",
                                     name="ktev", bufs=2)
                        nc.scalar.copy(kt, pa)
                        if s < 2:
                            pack_dma(kt, ktilT[s], oc)
                        if s == 0:
                            nc.vector.tensor_copy(kbu[oc], kt)
                        else:
                            nc.vector.tensor_add(kbu[oc], kbu[oc], kt)
                for oc in range(OC):
                    pack_dma(kbu[oc], kbarT, oc)

            # Q side: QbarT via PSUM accumulation over (t, kc), then rel_att fold
            qbarT = []
            with tc.tile_pool(name="psQ", bufs=2, space="PSUM") as psQ:
                for oc in range(OC):
                    pa = psQ.tile([128, I], dt.float32, tag="pQb", name="pQb")
                    for t_ in range(T):
                        xsi_t = [xim[0][:, t_ * I:(t_ + 1) * I],
                                 xim[1][:, t_ * I:(t_ + 1) * I],
                                 xim[2][0:1, t_ * I:(t_ + 1) * I]]
                        for kc in range(3):
                            nc.tensor.matmul(pa,
                                             wq_t[t_][kc][:, oc * 128:(oc + 1) * 128],
                                             xsi_t[kc], start=(t_ == 0 and kc == 0),
                                             stop=(t_ == T - 1 and kc == 2))
                    qb = sA.tile([128, I], dt.bfloat16, tag=f"qbar{oc}",
                                 name=f"qbar{oc}")
                    nc.scalar.copy(qb, pa)
                    qbarT.append(qb)
                for r in range(R):
                    qtilT.append(hpack_alloc(f"qtil{r}_", I))
                    for ot in range(OC):
                        pa = psQ.tile([128, I], dt.float32, tag="pQb", name="pQb")
                        for kc in range(OC):
                            nc.tensor.matmul(pa,
                                             abd_t[r][kc][:, ot * 128:(ot + 1) * 128],
                                             qbarT[kc], start=(kc == 0), stop=(kc == 1))
                        qt = sA.tile([128, I], dt.bfloat16, tag="qt128",
                                     name="qt128", bufs=2)
                        nc.vector.tensor_copy(qt, pa)
                        pack_dma(qt, qtilT[r], ot)

            # V side: V_2 and the differences V_s - V_2 (s=0,1)
            with tc.tile_pool(name="psV", bufs=3, space="PSUM") as psV:
                vtmp = [[None] * NJC for _ in range(2)]
                for s in range(T):
                    for jb in range(NJC):
                        pa = psV.tile([128, OUT], dt.float32, tag="pV", name="pV")
                        for kc in range(3):
                            nc.tensor.matmul(pa,
                                             xT_t[kc][:, jb * 128:(jb + 1) * 128],
                                             wv_t[s][kc], start=(kc == 0), stop=(kc == 2))
                        if s < 2:
                            v = sA.tile([128, OUT], dt.bfloat16, tag=f"vt{s}_{jb}",
                                        name=f"vt{s}_{jb}")
                            vtmp[s][jb] = v
                        else:
                            v = const.tile([128, OUT], dt.bfloat16, tag=f"v2_{jb}",
                                           name=f"v2_{jb}")
                            v2[jb] = v
                        nc.vector.tensor_copy(v, pa)
                for s in range(2):
                    for jb in range(NJC):
                        d = const.tile([128, OUT], dt.bfloat16, tag=f"vd{s}_{jb}",
                                       name=f"vd{s}_{jb}")
                        nc.vector.tensor_sub(d, vtmp[s][jb], v2[jb])
                        vd[s][jb] = d

        # precomputed mask pairs: mp[0][jc] = [m_0 | m_1], mp[1][jc] = [m_2 | m_3]
        mp = [[None] * NJC for _ in range(2)]
        mfull = [None] * NJC
        for jc in range(NJC):
            m4 = const.tile([128, 4 * I], dt.bfloat16, tag=f"mp_{jc}",
                            name=f"mp_{jc}")
            dma_q[jc % 2].dma_start(out=m4, in_=mpd[jc * 128:(jc + 1) * 128, :])
            mfull[jc] = m4
            for p in range(2):
                mp[p][jc] = m4[:, p * 2 * I:(p + 1) * 2 * I]
        xI4 = const.tile([128, NIB * IN], dt.bfloat16, tag="xI4", name="xI4")
        nc.sync.dma_start(out=xI4, in_=stack2(xI, 128, NIB, IN))
        xI_t = [xI4[:, ib * IN:(ib + 1) * IN] for ib in range(NIB)]
        tyI4 = const.tile([128, NIB * T], dt.float32, tag="tyI4", name="tyI4")
        nc.scalar.dma_start(out=tyI4, in_=stack2(typesI, 128, NIB, T))
        tyI_t = [tyI4[:, ib * T:(ib + 1) * T] for ib in range(NIB)]
        gBt = const.tile([128, T * OUT], dt.bfloat16, tag="gBt", name="gBt")
        nc.gpsimd.dma_start(out=gBt, in_=bass.AP(
            tensor=lng.tensor if hasattr(lng, 'tensor') else lng,
            offset=lng[0:1, :].offset,
            ap=[[0, 128]] + [list(x) for x in lng[0:T, :].ap]))
        gB = [gBt[:, t_ * OUT:(t_ + 1) * OUT] for t_ in range(T)]
        bBt = const.tile([128, T * OUT], dt.bfloat16, tag="bBt", name="bBt")
        nc.gpsimd.dma_start(out=bBt, in_=bass.AP(
            tensor=lnb.tensor if hasattr(lnb, 'tensor') else lnb,
            offset=lnb[0:1, :].offset,
            ap=[[0, 128]] + [list(x) for x in lnb[0:T, :].ap]))
        bB = [bBt[:, t_ * OUT:(t_ + 1) * OUT] for t_ in range(T)]
        alT = const.tile([128, 2 * T], dt.float32, tag="alT", name="alT")
        nc.gpsimd.dma_start(out=alT, in_=bass.AP(
            tensor=alph.tensor if hasattr(alph, 'tensor') else alph,
            offset=alph[0:1, :].offset,
            ap=[[0, 128]] + [list(x) for x in alph[0:2, :].ap]))
        alphB = [alT[:, t_:t_ + 1] for t_ in range(T)]
        alph1mB = [alT[:, T + t_:T + t_ + 1] for t_ in range(T)]
        wa_t = load_w3(const, wa, "wa", klen=OUT)
        bbd_c = []
        for kc in range(OC):
            tl = const.tile([128, R * OUT], dt.bfloat16, tag=f"bbdc{kc}",
                            name=f"bbdc{kc}")
            dma_q[kc % 2].dma_start(out=tl, in_=stack3(bbd, kc * 128, 128, R, OUT))
            bbd_c.append(tl)
        bbd_t = [[bbd_c[kc][:, r * OUT:(r + 1) * OUT] for kc in range(OC)]
                 for r in range(R)]

        # (1-alpha_t) * x residual tiles
        xa = [[None] * NIB for _ in range(T)]
        for t_ in range(T):
            for ib in range(NIB):
                tl = const.tile([128, IN], dt.bfloat16, tag=f"xa{t_}_{ib}",
                                name=f"xa{t_}_{ib}")
                nc.vector.tensor_scalar(out=tl, in0=xI_t[ib],
                                        scalar1=alph1mB[t_], scalar2=None,
                                        op0=ALU.mult)
                xa[t_][ib] = tl

        # ---------------- main h-loop: phase L / phase S per half-head -------
        # agg'_r / Z: one tile per target block, cols (r, h, 32) bf16
        aggn2 = []
        for ib in range(NIB):
            aggn2.append(const.tile([128, R * OUT], dt.bfloat16,
                                    tag=f"aggn{ib}", name=f"aggn{ib}"))

        def strided(t, col0, stride, n, w):
            a = t[:, col0:col0 + w]
            return bass.AP(tensor=a.tensor, offset=a.offset,
                           ap=[list(a.ap[0]), [stride, n]] + list(a.ap[1:]))

        GRP = 4  # jc per phase group (half-head)

        with tc.tile_pool(name="psL", bufs=1, space="PSUM") as psL, \
             tc.tile_pool(name="psM", bufs=2, space="PSUM") as psM, \
             tc.tile_pool(name="psG", bufs=1, space="PSUM") as psG, \
             tc.tile_pool(name="psZR", bufs=1, space="PSUM") as psZR, \
             tc.tile_pool(name="ph", bufs=10) as ph, \
             tc.tile_pool(name="xp", bufs=2) as xp:

            # one bank: zP double-buffered per head parity + global row-sum
            zr = psZR.tile([128, 3 * NIB], dt.float32, tag="zr", name="zr")
            pending_epilogue = [None]

            def head_epilogue(h, aggp):
                # normalize by 1/Z while evacuating this head's aggregation
                zo = (h % 2) * NIB
                zh = work.tile([128, NIB], dt.float32, tag="zh", name="zh")
                nc.vector.tensor_copy(zh, zr[:, zo:zo + NIB])
                zrec = work.tile([128, NIB], dt.float32, tag="zrec", name="zrec",
                                 bufs=2)
                nc.vector.reciprocal(zrec, zh)
                for ib in range(NIB):
                    nc.scalar.activation(
                        strided(aggn2[ib], h * 32, OUT, R, 32),
                        strided(aggp, ib * 32, NIB * 32, R, 32),
                        AF.Copy, scale=zrec[:, ib:ib + 1])

            # Rolling software pipeline over global slots k = h*NJC + jc:
            # phase L for slot k+LEAD is emitted alongside phase S for slot k,
            # so PE/DVE/ACT never drain at head or group boundaries.
            NSLOT = H * NJC
            LEAD = GRP
            Uh = {}
            Wh = {}
            Ph = {}
            P4h = {}
            aggps = {}

            def emit_L(k):
                h, jc = divmod(k, NJC)
                psA = psL.tile([128, 2 * I], dt.float32, tag="sL",
                               name="sL")
                for rr in range(2):
                    nc.tensor.matmul(psA[:, rr * I:(rr + 1) * I],
                                     hs(kbarT, h, jc * 128, (jc + 1) * 128),
                                     hs(qtilT[rr], h),
                                     start=True, stop=True)
                dL = work.tile([128, 2 * I], dt.bfloat16, tag="dL",
                               name="dL", bufs=2)
                nc.scalar.copy(dL, psA)
                u4 = ph.tile([128, 4 * I], dt.bfloat16, tag="U4",
                             name="U4", bufs=LEAD + 1)
                u0 = u4[:, 0:2 * I]
                nc.vector.tensor_mul(u0, dL, mp[0][jc])
                psB = psL.tile([128, 2 * I], dt.float32, tag="sL",
                               name="sL")
                for rr in range(2):
                    nc.tensor.matmul(psB[:, rr * I:(rr + 1) * I],
                                     hs(kbarT, h, jc * 128, (jc + 1) * 128),
                                     hs(qtilT[2 + rr], h),
                                     start=True, stop=True)
                u1 = u4[:, 2 * I:4 * I]
                nc.vector.tensor_mul(u1, psB, mp[1][jc])
                Uh[k] = u4
                rw = ph.tile([128, 2 * I], dt.bfloat16, tag="rw",
                             name="rw", bufs=2)
                nc.vector.tensor_add(rw, u0, u1)
                res = ph.tile([128, I], dt.bfloat16, tag="res", name="res",
                              bufs=2)
                nc.vector.tensor_add(res, rw[:, 0:I], rw[:, I:2 * I])
                for ib in range(NIB):
                    nc.tensor.matmul(zr[:, 2 * NIB + ib:2 * NIB + ib + 1],
                                     res[:, ib * 128:(ib + 1) * 128],
                                     onesCol,
                                     start=(h == 0 and jc == 0),
                                     stop=(h == H - 1 and jc == NJC - 1))
                P = ph.tile([128, I], dt.bfloat16, tag="P", name="P",
                            bufs=LEAD + 1)
                nc.scalar.activation(P, res, AF.Exp)
                Ph[k] = P
                zo = (h % 2) * NIB
                for ib in range(NIB):
                    nc.tensor.matmul(zr[:, zo + ib:zo + ib + 1],
                                     P[:, ib * 128:(ib + 1) * 128],
                                     onesCol,
                                     start=(jc == 0), stop=(jc == NJC - 1))
                p0 = P[:, 0:I]
                P4 = bass.AP(tensor=p0.tensor, offset=p0.offset,
                             ap=[list(p0.ap[0]), [0, 4]] + list(p0.ap[1:]))
                P4h[k] = P4
                # attention weights on GPSIMD, ready ahead of phase S
                w4 = ph.tile([128, 4 * I], dt.bfloat16, tag="W4",
                             name="W4", bufs=LEAD + 1)
                for p in range(2):
                    for rr in range(2):
                        q = p * 2 * I + rr * I
                        nc.gpsimd.tensor_mul(
                            w4[:, q:q + I], P,
                            mp[p][jc][:, rr * I:(rr + 1) * I])
                Wh[k] = w4

            def emit_S(k):
                h, jc = divmod(k, NJC)
                aggp = aggps[h]
                u4 = Uh.pop(k)
                xb4 = xp.tile([128, 4 * I], dt.bfloat16, tag="X4",
                              name="Xb", bufs=2)
                nc.vector.tensor_mul(xb4, u4, P4h.pop(k))
                Xq = [(xb4, 0), (xb4, 1)]
                for s in range(2):
                    d4 = work.tile([128, 4 * I], dt.bfloat16, tag="sev",
                                   name="sev", bufs=2)
                    for p in range(2):
                        ps = psM.tile([128, 2 * I], dt.float32, tag="sc",
                                      name="sc")
                        for rr in range(2):
                            nc.tensor.matmul(
                                ps[:, rr * I:(rr + 1) * I],
                                hs(ktilT[s], h, jc * 128, (jc + 1) * 128),
                                hs(qtilT[2 * p + rr], h),
                                start=True, stop=True)
                        nc.scalar.copy(d4[:, p * 2 * I:(p + 1) * 2 * I], ps)
                    x = xp.tile([128, 4 * I], dt.bfloat16,
                                tag="X", name=f"Xs{s}", bufs=3)
                    nc.vector.tensor_mul(x, d4, Wh[k])
                    Xq.append((x, 0))
                    Xq.append((x, 1))
                Wh.pop(k)
                Ph.pop(k)
                basis = [(Xq[0], Xq[1], v2), (Xq[2], Xq[3], vd[0]),
                         (Xq[4], Xq[5], vd[1])]
                for b, (xa_, xb_, Vb) in enumerate(basis):
                    for p, (xt, off) in enumerate((xa_, xb_)):
                        for rr in range(2):
                            r = 2 * p + rr
                            base = (off * 2 + rr) * I
                            for ib in range(NIB):
                                nc.tensor.matmul(
                                    aggp[:, (r * NIB + ib) * 32:(r * NIB + ib) * 32 + 32],
                                    xt[:, base + ib * 128:base + (ib + 1) * 128],
                                    Vb[jc][:, h * 32:h * 32 + 32],
                                    start=(jc == 0 and b == 0),
                                    stop=(jc == NJC - 1 and b == 2))

            for k in range(NSLOT + LEAD):
                if k < NSLOT:
                    emit_L(k)
                s = k - LEAD
                if s >= 0:
                    h, jc = divmod(s, NJC)
                    if jc == 0:
                        if h > 0:
                            head_epilogue(h - 1, aggps.pop(h - 1))
                        aggps[h] = psG.tile([128, 512], dt.float32,
                                            tag="aggp", name="aggp")
                    emit_S(s)

            # flush the last head's epilogue
            head_epilogue(H - 1, aggps.pop(H - 1))
            # row-sum gate: zero rows whose logit-sum <= 1e-6
            rs_sb = work.tile([128, NIB], dt.float32, tag="rssb", name="rssb")
            nc.vector.tensor_copy(rs_sb, zr[:, 2 * NIB:3 * NIB])
            cond = const.tile([128, NIB], dt.float32, tag="cond", name="cond")
            nc.vector.tensor_scalar(out=cond, in0=rs_sb, scalar1=1e-6,
                                    scalar2=None, op0=ALU.is_gt)

        # ---------------- stage D: rel_msg, gelu, linear, LN, type mix ----------------
        with tc.tile_pool(name="psT", bufs=2, space="PSUM") as psT, \
             tc.tile_pool(name="psB", bufs=2, space="PSUM") as psB, \
             tc.tile_pool(name="psO", bufs=2, space="PSUM") as psO:
            # apply gate, transpose agg'_r -> [(h,d'), i]
            aggTin = [[None] * OC for _ in range(R)]
            for r in range(R):
                for kc in range(OC):
                    aggTin[r][kc] = const.tile([128, I], dt.bfloat16,
                                               tag=f"aggT{r}_{kc}",
                                               name=f"aggT{r}_{kc}")
            for r in range(R):
                for ib in range(NIB):
                    ac = work.tile([128, OUT], dt.bfloat16, tag="aggc", name="aggc",
                                   bufs=2)
                    nc.vector.tensor_scalar(out=ac,
                                            in0=aggn2[ib][:, r * OUT:(r + 1) * OUT],
                                            scalar1=cond[:, ib:ib + 1],
                                            scalar2=None, op0=ALU.mult)
                    for kc in range(OC):
                        pt = psT.tile([128, 128], dt.bfloat16, tag="tp", name="tp")
                        nc.tensor.transpose(pt, ac[:, kc * 128:(kc + 1) * 128],
                                            ident)
                        nc.scalar.copy(aggTin[r][kc][:, ib * 128:(ib + 1) * 128], pt)
            # aggT = sum_r Bbd_r-rotated aggregate, then gelu -> gT
            gT = []
            for ot in range(OC):
                pb = psB.tile([128, I], dt.float32, tag="aggTp", name="aggTp")
                for r in range(R):
                    for kc in range(OC):
                        nc.tensor.matmul(pb,
                                         bbd_t[r][kc][:, ot * 128:(ot + 1) * 128],
                                         aggTin[r][kc],
                                         start=(r == 0 and kc == 0),
                                         stop=(r == R - 1 and kc == OC - 1))
                g = const.tile([128, I], dt.bfloat16, tag=f"gT{ot}", name=f"gT{ot}")
                nc.scalar.activation(g, pb, AF.Gelu)
                gT.append(g)
            # per-type head: trans = g @ Wa_t + ba_t ; blend ; LN ; type mix
            for ib in range(NIB):
                o_prev = None
                for t_ in range(T):
                    po = psO.tile([128, OUT], dt.float32, tag="trp", name="trp")
                    for kc in range(OC):
                        nc.tensor.matmul(po, gT[kc][:, ib * 128:(ib + 1) * 128],
                                         wa_t[t_][kc], start=(kc == 0), stop=False)
                    nc.tensor.matmul(po, onesRow[0:1, ib * 128:(ib + 1) * 128],
                                     wa_t[t_][2], start=False, stop=True)
                    rt = work.tile([128, OUT], dt.bfloat16, tag="rt", name="rt",
                                   bufs=2)
                    nc.vector.scalar_tensor_tensor(out=rt, in0=po,
                                                   scalar=alphB[t_], in1=xa[t_][ib],
                                                   op0=ALU.mult, op1=ALU.add)
                    st6 = work.tile([128, 6], dt.float32, tag="st6", name="st6",
                                    bufs=2)
                    nc.vector.bn_stats(out=st6, in_=rt)
                    mv = work.tile([128, 2], dt.float32, tag="mv", name="mv",
                                   bufs=2)
                    nc.vector.bn_aggr(out=mv, in_=st6)
                    ve = work.tile([128, 1], dt.float32, tag="ve", name="ve",
                                   bufs=2)
                    nc.vector.tensor_scalar(out=ve, in0=mv[:, 1:2], scalar1=LN_EPS,
                                            scalar2=None, op0=ALU.add)
                    sd = work.tile([128, 1], dt.float32, tag="sd", name="sd",
                                   bufs=2)
                    nc.scalar.sqrt(sd, ve)
                    rstd = work.tile([128, 1], dt.float32, tag="rstd", name="rstd",
                                     bufs=2)
                    nc.vector.reciprocal(rstd, sd)
                    cen = work.tile([128, OUT], dt.bfloat16, tag="cen", name="cen",
                                    bufs=2)
                    nc.vector.tensor_scalar(out=cen, in0=rt, scalar1=mv[:, 0:1],
                                            scalar2=None, op0=ALU.subtract)
                    v1 = work.tile([128, OUT], dt.bfloat16, tag="v1", name="v1",
                                   bufs=2)
                    nc.vector.scalar_tensor_tensor(out=v1, in0=cen, scalar=rstd,
                                                   in1=gB[t_], op0=ALU.mult,
                                                   op1=ALU.mult)
                    tycol = tyI_t[ib][:, t_:t_ + 1]
                    if t_ == 0:
                        ob = work.tile([128, OUT], dt.bfloat16, tag="ob0",
                                       name="ob0", bufs=2)
                        nc.vector.tensor_scalar(out=ob, in0=bB[t_], scalar1=tycol,
                                                scalar2=None, op0=ALU.mult)
                    else:
                        ob = work.tile([128, OUT], dt.bfloat16, tag=f"ob{t_}",
                                       name=f"ob{t_}", bufs=2)
                        nc.vector.scalar_tensor_tensor(out=ob, in0=bB[t_],
                                                       scalar=tycol, in1=o_prev,
                                                       op0=ALU.mult, op1=ALU.add)
                    odt = dt.float32 if t_ == T - 1 else dt.bfloat16
                    onew = work.tile([128, OUT], odt, tag=f"oacc{t_}",
                                     name=f"oacc{t_}", bufs=2)
                    nc.vector.scalar_tensor_tensor(out=onew, in0=v1, scalar=tycol,
                                                   in1=ob, op0=ALU.mult, op1=ALU.add)
                    o_prev = onew
                nc.sync.dma_start(out=outP[ib * 128:(ib + 1) * 128, :], in_=o_prev)

    nc.compile()
    _built["nc"] = nc
    return nc


def _host_prep(inputs):
    """Build the 8 per-core input dicts (numpy, host-side sharding/casts)."""
    f32 = np.float32
    x_all = np.asarray(inputs["node_features"], f32)
    ty_all = np.asarray(inputs["node_types_soft"], f32)
    adj_all = np.asarray(inputs["adj_matrix_soft"], f32)
    erel_all = np.asarray(inputs["edge_types_soft"], f32)
    Wq = np.asarray(inputs["Wq"], f32); bq = np.asarray(inputs["bq"], f32)
    Wk = np.asarray(inputs["Wk"], f32); bk = np.asarray(inputs["bk"], f32)
    Wv = np.asarray(inputs["Wv"], f32); bv = np.asarray(inputs["bv"], f32)
    Wa = np.asarray(inputs["Wa"], f32); ba = np.asarray(inputs["ba"], f32)
    rel_pri = np.asarray(inputs["rel_pri"], f32)
    rel_att = np.asarray(inputs["rel_att"], f32)
    rel_msg = np.asarray(inputs["rel_msg"], f32)
    skip = np.asarray(inputs["skip"], f32)
    lng = np.asarray(inputs["ln_gamma"], f32)
    lnb = np.asarray(inputs["ln_beta"], f32)

    sqrt_dk = math.sqrt(DK)
    abd = np.zeros((R, OUT, OUT), f32)
    bbd = np.zeros((R, OUT, OUT), f32)
    for r in range(R):
        for h in range(H):
            sl = slice(h * DK, (h + 1) * DK)
            abd[r, sl, sl] = rel_att[r, h].T * (rel_pri[r, h] / sqrt_dk)
            bbd[r, sl, sl] = rel_msg[r, h]
    alpha = 1.0 / (1.0 + np.exp(-skip))
    alph = np.stack([alpha, 1.0 - alpha]).astype(f32)

    def bf(a):
        return np.ascontiguousarray(a.astype(BF16))

    wq_aug = bf(np.concatenate([Wq, bq[:, None, :]], axis=1))
    wk_aug = bf(np.concatenate([Wk, bk[:, None, :]], axis=1))
    wv_aug = bf(np.concatenate([Wv, bv[:, None, :]], axis=1))
    wa_aug = bf(np.concatenate([Wa, ba[:, None, :]], axis=1))
    abd_b, bbd_b = bf(abd), bf(bbd)
    lng_b, lnb_b = bf(lng), bf(lnb)

    in_maps = []
    for c in range(NCORES):
        b, half = c // 2, c % 2
        isl = slice(half * I, half * I + I)
        x = x_all[b]
        ty = ty_all[b]
        xT_aug = np.concatenate([x.T, np.ones((1, N), f32)], axis=0)
        xTs_aug = np.stack([
            np.concatenate([(x * ty[:, s:s + 1]).T, ty[None, :, s]], axis=0)
            for s in range(T)])
        # mask pairs m_r^T = (adj . erel_r)^T, packed [m_0|m_1] and [m_2|m_3]
        adjTc = adj_all[b][isl, :].T                      # [N, I]
        erelTc = erel_all[b][isl, :, :]                   # [I, N, R]
        mT = adjTc[:, None, :] * erelTc.transpose(1, 2, 0)  # [N, R, I]
        mpair = np.concatenate([mT[:, 0, :], mT[:, 1, :],
                                mT[:, 2, :], mT[:, 3, :]], axis=1)
        in_maps.append({
            "xT": bf(xT_aug),
            "xI": bf(x[isl]),
            "xTs": bf(xTs_aug),
            "xTsI": bf(xTs_aug[:, :, isl]),
            "typesI": np.ascontiguousarray(ty[isl]),
            "mpair": bf(mpair),
            "wq": wq_aug, "wk": wk_aug, "wv": wv_aug, "wa": wa_aug,
            "abd": abd_b, "bbd": bbd_b, "lng": lng_b, "lnb": lnb_b,
            "alph": alph,
        })
    return in_maps


def kernel(**inputs):
    from concourse.bass_utils import run_bass_kernel_spmd
    nc = _build_nc()
    in_maps = _host_prep(inputs)
    res = None
    for attempt in range(3):
        try:
            res = run_bass_kernel_spmd(nc, in_maps, core_ids=list(range(NCORES)))
            break
        except Exception:
            # transient accelerator/tunnel failures recover on retry
            if attempt == 2:
                raise
            import time
            time.sleep(10)
    out = np.zeros((B, N, OUT), np.float32)
    for c in range(NCORES):
        b, half = c // 2, c % 2
        out[b, half * I:half * I + I, :] = res.results[c]["out"]
    return out

